# revision 24
# baseline (speedup 1.0000x reference)
"""Causal self-attention on 8 TRN2 NeuronCores, batch-data-parallel (one batch
element per core).

bf16 matmul operands (PSUM accum fp32; softmax recip path fp32), all weights
SBUF-resident (single DMA pass, interleaved fine-grained so the first QK
matmul unblocks a few us in), score-pair matmuls emitted adjacently so the
two K=64 heads run concurrently on PE row groups 0-63/64-127, AV matmuls
narrowed to the causally-nonzero column range (no zero-fill pass), projection
bias via DVE add of a host-broadcast bias tile, per-head normalization
multiplied directly from the rank-1 PSUM broadcast, projection chains for
ko 0..6 pre-started before the last pair's eviction epilogue so the PE stays
busy (and HAM stays warm) across the attention->projection transition, and
y DMAs split 4-way so the last tile lands ~3us after its matmul.

Layout (per core, S=1024, D=1024, H=16, hd=64):
  - Host pre-transposes x -> xT [D,S] bf16, weights -> [in,out] bf16.
  - q,k produced transposed ([e,s]) per head-pair; head h at partitions
    64*(h%2)..+64 of qk tile j=h//2.
  - v natural [s,e] interleaved with a ones column per head (65 cols/head) so
    AV's PSUM row 64 is the softmax denominator.
  - scoresT [sk,sq] per head-pair; exp on ACT (scale 1/8 folded); causal diag
    masked multiplicatively; fully-masked tiles never computed.
  - AV accumulated m-major into [65,1024] PSUM; normalization = fp32
    approx-reciprocal of den row + PE rank-1 broadcast + DVE multiply.
  - QKV matmul chains interleave into the attention pair loop through a
    dedicated 1-bank PSUM pool so they fill PE gaps while ACT runs exp.
"""

import numpy as np

B, S, D, H = 8, 1024, 1024, 16
HD = D // H          # 64
P = 128
NCORES = 8
KO = D // P          # 8 contraction tiles over d
ST = S // P          # 8 s-tiles
NPAIRS = H // 2      # 8 head pairs

_CACHE = {}
TRACE = False        # set by test harness to collect an NTFF profile


def _score_chunks(w):
    # pieces <=512 (PSUM bank limit); bf16 streams full-rate at any width
    table = {1024: [512, 512], 896: [512, 384], 768: [512, 256],
             640: [384, 256], 512: [512], 384: [384], 256: [256], 128: [128]}
    return table[w]


def _build():
    import concourse.tile as tile
    from concourse import bacc, mybir

    BF = mybir.dt.bfloat16
    F32R = mybir.dt.float32r
    F32 = mybir.dt.float32
    AF = mybir.ActivationFunctionType

    nc = bacc.Bacc("TRN2", target_bir_lowering=False, debug=False,
                   num_devices=NCORES)
    xT_d = nc.dram_tensor("xT", [D, S], BF, kind="ExternalInput").ap()
    wqkT_d = nc.dram_tensor("wqkT", [D, 2 * D], BF, kind="ExternalInput").ap()
    wvT_d = nc.dram_tensor("wvT", [D, D], BF, kind="ExternalInput").ap()
    wpT_d = nc.dram_tensor("wpT", [D, D], BF, kind="ExternalInput").ap()
    bqk_d = nc.dram_tensor("bqk", [2 * D], F32, kind="ExternalInput").ap()
    beffb_d = nc.dram_tensor("beffb", [P, D], F32, kind="ExternalInput").ap()
    umask_d = nc.dram_tensor("umask", [P, P], BF, kind="ExternalInput").ap()
    y_d = nc.dram_tensor("y", [S, D], F32, kind="ExternalOutput").ap()

    wqkT_v = wqkT_d.rearrange("(ko p) e -> p ko e", p=P)
    wvT_v = wvT_d.rearrange("(ko p) e -> p ko e", p=P)
    wpT_v = wpT_d.rearrange("(ko p) e -> p ko e", p=P)
    xT_v = xT_d.rearrange("(ko p) s -> p ko s", p=P)

    with tile.TileContext(nc) as tc:
        with (
            tc.tile_pool(name="bigio", bufs=1) as bigio,
            tc.tile_pool(name="qkp", bufs=3) as qkp,
            tc.tile_pool(name="vp", bufs=1) as vpool,
            tc.tile_pool(name="attn", bufs=8) as attnp,
            tc.tile_pool(name="rt", bufs=2) as rtp,
            tc.tile_pool(name="todd", bufs=1) as toddp,
            tc.tile_pool(name="ystg", bufs=2) as ystgp,
            tc.tile_pool(name="avsb", bufs=2) as avsbp,
            tc.tile_pool(name="cst", bufs=1) as cst,
            tc.tile_pool(name="psS", bufs=3, space="PSUM") as psS,
            tc.tile_pool(name="psQ", bufs=1, space="PSUM") as psQ,
            tc.tile_pool(name="psAV", bufs=2, space="PSUM") as psAV,
        ):
            # ---------- small constants ----------
            umask = cst.tile([P, P], BF)
            nc.sync.dma_start(umask[:], umask_d)
            bqk_sb = cst.tile([P, 2 * D // P], F32)
            nc.sync.dma_start(bqk_sb[:], bqk_d.rearrange("(m p) -> p m", p=P))
            onecol = cst.tile([P, 1], BF)
            nc.vector.memset(onecol[:], 1.0)
            of32 = cst.tile([65, 64], F32)
            nc.vector.memset(of32[64:65, :], 1.0)
            ones_r = cst.tile([65, 64], F32R)
            nc.vector.tensor_copy(ones_r[64:65, :], of32[64:65, :])

            # ---------- big SBUF residents (DMA priority order) ----------
            xT = bigio.tile([P, KO, S], BF, tag="xT")
            wqk_sb = bigio.tile([P, KO, 2 * D], BF, tag="wqk")
            # startup-critical transfers split fine and interleaved so the
            # first QK chains unblock within a few us: quarters 0 (q m-tiles
            # 0-3) and 2 (k m-tiles 8-11) of wqk feed qk tiles 0,1
            for ko in range(KO):
                for q in (0, 2):
                    nc.sync.dma_start(
                        wqk_sb[:, ko, q * 512:(q + 1) * 512],
                        wqkT_v[:, ko, q * 512:(q + 1) * 512])
                for h in (0, 1):
                    nc.sync.dma_start(
                        xT[:, ko, h * 512:(h + 1) * 512],
                        xT_v[:, ko, h * 512:(h + 1) * 512])
            wv_sb = bigio.tile([P, KO, D], BF, tag="wv")
            for ko in range(KO):
                nc.sync.dma_start(wv_sb[:, ko, 0:512], wvT_v[:, ko, 0:512])
            for q in (1, 3):
                for ko in range(KO):
                    nc.sync.dma_start(
                        wqk_sb[:, ko, q * 512:(q + 1) * 512],
                        wqkT_v[:, ko, q * 512:(q + 1) * 512])
            for ko in range(KO):
                nc.sync.dma_start(wv_sb[:, ko, 512:1024],
                                  wvT_v[:, ko, 512:1024])
            wp_sb = bigio.tile([P, KO, D], BF, tag="wp")
            for ko in range(KO):
                nc.sync.dma_start(wp_sb[:, ko, :], wpT_v[:, ko, :])
            beffb = cst.tile([P, D], F32)
            nc.sync.dma_start(beffb[:], beffb_d)

            outT = bigio.tile([P, KO, S], BF, tag="outT")
            v_sb = vpool.tile([P, ST, H * (HD + 1)], BF)
            v_hview = v_sb[:].rearrange("p st (h c) -> p st h c", c=HD + 1)
            nc.vector.tensor_copy(
                v_hview[:, :, :, HD:HD + 1],
                onecol[:, None, None, :].broadcast_to([P, ST, H, 1]))

            qk_tiles = {}    # j -> [128, 2, S] tile (0=q, 1=k)

            # ---------- QKV work chains (each: 8 matmuls, 1 PSUM bank) ----
            def qk_chain(j, part, nn):   # part 0=q (m-tile j), 1=k (8+j)
                def go():
                    m = j if part == 0 else NPAIRS + j
                    t = qk_tiles[j]
                    ps = psQ.tile([P, 512], F32, tag="psq",
                                  name=f"qkps{m}_{nn}")
                    for ko in range(KO):
                        nc.tensor.matmul(
                            ps[:], wqk_sb[:, ko, m * P:(m + 1) * P],
                            xT[:, ko, nn * 512:(nn + 1) * 512],
                            start=(ko == 0), stop=(ko == KO - 1))
                    nc.vector.tensor_scalar_add(
                        t[:, part, nn * 512:(nn + 1) * 512], ps[:],
                        bqk_sb[:, m:m + 1])
                return go

            def qk_alloc(j):
                qk_tiles[j] = qkp.tile([P, 2, S], BF, tag="qkt",
                                       name=f"qk{j}")
                return [qk_chain(j, part, nn)
                        for part in (0, 1) for nn in (0, 1)]

            def v_chain(st, nE):
                def go():
                    ps = psQ.tile([P, 512], F32, tag="psq",
                                  name=f"vps{nE}_{st}")
                    for ko in range(KO):
                        nc.tensor.matmul(
                            ps[:], xT[:, ko, st * P:(st + 1) * P],
                            wv_sb[:, ko, nE * 512:(nE + 1) * 512],
                            start=(ko == 0), stop=(ko == KO - 1))
                    nc.vector.tensor_copy(
                        v_hview[:, st, 8 * nE:8 * (nE + 1), 0:HD],
                        ps[:].rearrange("p (h c) -> p h c", c=HD))
                return go

            # ---------- attention ----------
            pend = {}

            def scores_exp(j, m):
                # emit the two heads' matmuls adjacently per chunk so they
                # run concurrently on PE row groups 0-63 / 64-127
                qk_t = qk_tiles[j]
                w = S - m * P
                ats = []
                for hb, base in ((0, 0), (1, 64)):
                    at = attnp.tile([P, S], BF, tag="at",
                                    name=f"at{j}_{hb}_{m}")
                    pend[(j, hb, m)] = at
                    ats.append(at)
                off = m * P
                for cw in _score_chunks(w):
                    pss = []
                    for hb, base in ((0, 0), (1, 64)):
                        ps = psS.tile([P, 512], F32, tag="ps",
                                      name=f"sps{j}_{hb}_{m}")
                        nc.tensor.matmul(
                            ps[:, 0:cw],
                            qk_t[base:base + 64, 1, m * P:(m + 1) * P],
                            qk_t[base:base + 64, 0, off:off + cw],
                            start=True, stop=True)
                        pss.append(ps)
                    for hb in (0, 1):
                        nc.scalar.activation(
                            ats[hb][:, off:off + cw], pss[hb][:, 0:cw],
                            AF.Exp, scale=0.125)
                    off += cw
                for hb in (0, 1):
                    nc.vector.tensor_mul(
                        ats[hb][:, m * P:(m + 1) * P],
                        ats[hb][:, m * P:(m + 1) * P], umask[:])

            def av_m(j, m):
                # narrowed to the causally-nonzero range of each 512 chunk;
                # partial-width accumulate is element-wise legal on HW
                st8 = pend[f"ps{j}"]
                for hb in (0, 1):
                    h = 2 * j + hb
                    at = pend[(j, hb, m)]
                    for n in range(2):
                        lo = max(n * 512, m * P)
                        hi = (n + 1) * 512
                        if lo >= hi:
                            continue
                        nc.tensor.matmul(
                            st8[hb][:, lo:hi],
                            v_sb[:, m, h * (HD + 1):(h + 1) * (HD + 1)],
                            at[:, lo:hi],
                            start=(m == 0), stop=(m == 4 * n + 3),
                            skip_group_check=True)

            def evict_recip(j):
                # move [65,S] AV accumulators out of PSUM so the next pair's
                # AV matmuls get the banks; reciprocal of den row in fp32
                avcs, recs = [], []
                for hb in (0, 1):
                    avc = avsbp.tile([65, S], F32, tag="avc",
                                     name=f"avc{j}_{hb}")
                    nc.vector.tensor_copy(avc[:], pend[f"ps{j}"][hb][:])
                    avcs.append(avc)
                    rt = rtp.tile([65, S], F32R, tag="rt")
                    rt32 = rtp.tile([65, S], F32, tag="rt32", bufs=1)
                    # custom-DVE op misbehaves on single-partition APs on HW:
                    # run over all 65 rows, consume only the den row (64)
                    nc.vector.reciprocal_approx_fast(rt32[:], avc[:])
                    nc.vector.tensor_copy(rt[64:65, :], rt32[64:65, :])
                    recs.append(rt)
                pend[f"avc{j}"] = avcs
                pend[f"rec{j}"] = recs
                del pend[f"ps{j}"]

            def rb_norm(j):
                for hb in (0, 1):
                    rt = pend[f"rec{j}"][hb]
                    avc = pend[f"avc{j}"][hb]
                    tmp = None
                    if hb == 1:
                        tmp = toddp.tile([64, S], BF, tag="todd")
                    for c in range(2):
                        rps = psS.tile([64, 512], F32, tag="ps",
                                       name=f"rbps{j}_{hb}_{c}")
                        nc.tensor.matmul(
                            rps[:], ones_r[64:65, :],
                            rt[64:65, c * 512:(c + 1) * 512],
                            start=True, stop=True)
                        if hb == 0:
                            nc.vector.tensor_mul(
                                outT[0:64, j, c * 512:(c + 1) * 512],
                                avc[0:64, c * 512:(c + 1) * 512], rps[:])
                        else:
                            # DVE lanes cannot shift partitions: multiply to
                            # SBUF tmp, DMA-shift rows 0..63 -> 64..127
                            nc.vector.tensor_mul(
                                tmp[:, c * 512:(c + 1) * 512],
                                avc[0:64, c * 512:(c + 1) * 512], rps[:])
                    if hb == 1:
                        for c in range(2):
                            nc.sync.dma_start(
                                outT[64:128, j, c * 512:(c + 1) * 512],
                                tmp[:, c * 512:(c + 1) * 512])
                del pend[f"avc{j}"], pend[f"rec{j}"]

            def proj_evict(ps, st, nE):
                ystg = ystgp.tile([P, 512], F32, tag="ystg",
                                  name=f"ystg{st}")
                nc.vector.tensor_add(
                    ystg[:], ps[:], beffb[:, nE * 512:(nE + 1) * 512])
                # 4-way split so the last y tile lands ~3us after its
                # matmul instead of ~12us (256KB on a single queue)
                for part in range(4):
                    nc.sync.dma_start(
                        y_d[st * P:(st + 1) * P,
                            nE * 512 + part * 128:nE * 512 + (part + 1) * 128],
                        ystg[:, part * 128:(part + 1) * 128])

            # ---------- interleaved emission ----------
            # prologue: qk for pairs 0,1 and v half 0
            for c in qk_alloc(0):
                c()
            for c in qk_alloc(1):
                c()
            for st in range(ST):
                v_chain(st, 0)()
            vwork = [v_chain(st, 1) for st in range(ST)]

            for j in range(NPAIRS):
                work = []
                if j + 2 < NPAIRS:
                    work.extend(qk_alloc(j + 2))
                if j < 4 and vwork:
                    work.append(vwork.pop(0))
                    work.append(vwork.pop(0))
                for m in range(ST):
                    scores_exp(j, m)
                    if m == 4 and j > 0:
                        rb_norm(j - 1)
                    if m == 0:
                        pend[f"ps{j}"] = [
                            psAV.tile([65, S], F32, tag="av",
                                      name=f"av{j}_{hb}") for hb in range(2)]
                    if m >= 2:
                        av_m(j, m - 2)
                    if work:
                        work.pop(0)()
                av_m(j, ST - 2)
                while work:
                    work.pop(0)()
                av_m(j, ST - 1)
                if j == NPAIRS - 1:
                    # pre-start proj chains (ko 0..6 need only pairs 0..6)
                    # to keep PE busy through the last pair's epilogue
                    pre = []
                    for st, pool, tg in ((0, psQ, "psq"), (1, psS, "ps"),
                                         (2, psS, "ps")):
                        ps = pool.tile([P, 512], F32, tag=tg,
                                       name=f"ypre{st}")
                        for ko in range(KO - 1):
                            nc.tensor.matmul(
                                ps[:], outT[:, ko, st * P:(st + 1) * P],
                                wp_sb[:, ko, 0:512],
                                start=(ko == 0), stop=False)
                        pre.append(ps)
                evict_recip(j)
            rb_norm(NPAIRS - 1)

            # ---------- output projection ----------
            for st in (0, 1, 2):
                nc.tensor.matmul(
                    pre[st][:], outT[:, KO - 1, st * P:(st + 1) * P],
                    wp_sb[:, KO - 1, 0:512], start=False, stop=True)
                proj_evict(pre[st], st, 0)
            groups = [((0, 1, 2), 1), ((3, 4, 5), 0), ((3, 4, 5), 1),
                      ((6, 7), 0), ((6, 7), 1)]
            for sts, nE in groups:
                pss = {st: psS.tile([P, 512], F32, tag="ps",
                                    name=f"yps{st}") for st in sts}
                for ko in range(KO):
                    for st in sts:
                        nc.tensor.matmul(
                            pss[st][:],
                            outT[:, ko, st * P:(st + 1) * P],
                            wp_sb[:, ko, nE * 512:(nE + 1) * 512],
                            start=(ko == 0), stop=(ko == KO - 1))
                for st in sts:
                    proj_evict(pss[st], st, nE)

    nc.compile()
    return nc


def kernel(x, w_attn, b_attn, w_proj, b_proj):
    import concourse.bass_utils as bass_utils
    import ml_dtypes

    if "nc" not in _CACHE:
        _CACHE["nc"] = _build()
    nc = _CACHE["nc"]

    bf16 = ml_dtypes.bfloat16
    x = np.asarray(x, dtype=np.float32)
    w_attn = np.asarray(w_attn, dtype=np.float32)
    b_attn = np.asarray(b_attn, dtype=np.float32)
    w_proj = np.asarray(w_proj, dtype=np.float32)
    b_proj = np.asarray(b_proj, dtype=np.float32)

    xT = np.ascontiguousarray(
        np.transpose(x, (0, 2, 1))).astype(bf16)                 # [B, D, S]
    wqkT = np.ascontiguousarray(w_attn[:2 * D].T).astype(bf16)   # [D, 2D]
    wvT = np.ascontiguousarray(w_attn[2 * D:].T).astype(bf16)    # [D, D]
    wpT = np.ascontiguousarray(w_proj.T).astype(bf16)            # [D, D]
    bqk = np.ascontiguousarray(b_attn[:2 * D])
    bv = b_attn[2 * D:]
    beff = (b_proj.astype(np.float64)
            + w_proj.astype(np.float64) @ bv.astype(np.float64)
            ).astype(np.float32)
    beffb = np.ascontiguousarray(np.broadcast_to(beff, (P, D)))
    umask = np.triu(np.ones((P, P), dtype=np.float32)).astype(bf16)

    in_maps = [
        dict(xT=xT[b], wqkT=wqkT, wvT=wvT, wpT=wpT, bqk=bqk, beffb=beffb,
             umask=umask)
        for b in range(B)
    ]
    res = bass_utils.run_bass_kernel_spmd(
        nc, in_maps, core_ids=list(range(NCORES)), trace=TRACE)
    if TRACE:
        _CACHE["exec_time_ns"] = res.exec_time_ns
        _CACHE["trace"] = res.instructions_and_trace
    return np.stack([res.results[b]["y"] for b in range(B)], axis=0)


# revision 28
# speedup vs baseline: 1.1208x; 1.1208x over previous
"""Causal self-attention on 8 TRN2 NeuronCores, batch-data-parallel (one batch
element per core).

bf16 matmul operands (PSUM accum fp32; softmax recip path fp32), all weights
SBUF-resident (single DMA pass, interleaved fine-grained so the first QK
matmul unblocks a few us in), score-pair matmuls emitted adjacently so the
two K=64 heads run concurrently on PE row groups 0-63/64-127, AV matmuls
narrowed to the causally-nonzero column range (no zero-fill pass), projection
bias via DVE add of a host-broadcast bias tile, per-head normalization
multiplied directly from the rank-1 PSUM broadcast, projection chains for
ko 0..6 pre-started before the last pair's eviction epilogue so the PE stays
busy (and HAM stays warm) across the attention->projection transition, and
y DMAs split 4-way so the last tile lands ~3us after its matmul.

Layout (per core, S=1024, D=1024, H=16, hd=64):
  - Host pre-transposes x -> xT [D,S] bf16, weights -> [in,out] bf16.
  - q,k produced transposed ([e,s]) per head-pair; head h at partitions
    64*(h%2)..+64 of qk tile j=h//2.
  - v natural [s,e] interleaved with a ones column per head (65 cols/head) so
    AV's PSUM row 64 is the softmax denominator.
  - scoresT [sk,sq] per head-pair; exp on ACT (scale 1/8 folded); causal diag
    masked multiplicatively; fully-masked tiles never computed.
  - AV accumulated m-major into [65,1024] PSUM; normalization = fp32
    approx-reciprocal of den row + PE rank-1 broadcast + DVE multiply.
  - QKV matmul chains interleave into the attention pair loop through a
    dedicated 1-bank PSUM pool so they fill PE gaps while ACT runs exp.
"""

import numpy as np

B, S, D, H = 8, 1024, 1024, 16
HD = D // H          # 64
P = 128
NCORES = 8
KO = D // P          # 8 contraction tiles over d
ST = S // P          # 8 s-tiles
NPAIRS = H // 2      # 8 head pairs

_CACHE = {}
TRACE = False        # set by test harness to collect an NTFF profile


def _score_chunks(w):
    # pieces <=512 (PSUM bank limit); bf16 streams full-rate at any width
    table = {1024: [512, 512], 896: [512, 384], 768: [512, 256],
             640: [384, 256], 512: [512], 384: [384], 256: [256], 128: [128]}
    return table[w]


def _build():
    import concourse.tile as tile
    from concourse import bacc, mybir

    BF = mybir.dt.bfloat16
    F32R = mybir.dt.float32r
    F32 = mybir.dt.float32
    AF = mybir.ActivationFunctionType

    nc = bacc.Bacc("TRN2", target_bir_lowering=False, debug=False,
                   num_devices=NCORES)
    xT_d = nc.dram_tensor("xT", [D, S], BF, kind="ExternalInput").ap()
    wqkT_d = nc.dram_tensor("wqkT", [D, 2 * D], BF, kind="ExternalInput").ap()
    wvT_d = nc.dram_tensor("wvT", [D, D], BF, kind="ExternalInput").ap()
    wpT_d = nc.dram_tensor("wpT", [D, D], BF, kind="ExternalInput").ap()
    bqk_d = nc.dram_tensor("bqk", [2 * D], F32, kind="ExternalInput").ap()
    beffb_d = nc.dram_tensor("beffb", [P, D], F32, kind="ExternalInput").ap()
    umask_d = nc.dram_tensor("umask", [P, P], BF, kind="ExternalInput").ap()
    y_d = nc.dram_tensor("y", [S, D], F32, kind="ExternalOutput").ap()

    wqkT_v = wqkT_d.rearrange("(ko p) e -> p ko e", p=P)
    wvT_v = wvT_d.rearrange("(ko p) e -> p ko e", p=P)
    wpT_v = wpT_d.rearrange("(ko p) e -> p ko e", p=P)
    xT_v = xT_d.rearrange("(ko p) s -> p ko s", p=P)

    with tile.TileContext(nc) as tc:
        with (
            tc.tile_pool(name="bigio", bufs=1) as bigio,
            tc.tile_pool(name="qkp", bufs=3) as qkp,
            tc.tile_pool(name="vp", bufs=1) as vpool,
            tc.tile_pool(name="attn", bufs=8) as attnp,
            tc.tile_pool(name="rt", bufs=2) as rtp,
            tc.tile_pool(name="todd", bufs=1) as toddp,
            tc.tile_pool(name="ystg", bufs=2) as ystgp,
            tc.tile_pool(name="avsb", bufs=2) as avsbp,
            tc.tile_pool(name="cst", bufs=1) as cst,
            tc.tile_pool(name="psS", bufs=3, space="PSUM") as psS,
            tc.tile_pool(name="psQ", bufs=1, space="PSUM") as psQ,
            tc.tile_pool(name="psAV", bufs=2, space="PSUM") as psAV,
        ):
            # ---------- small constants ----------
            umask = cst.tile([P, P], BF)
            nc.sync.dma_start(umask[:], umask_d)
            bqk_sb = cst.tile([P, 2 * D // P], F32)
            nc.sync.dma_start(bqk_sb[:], bqk_d.rearrange("(m p) -> p m", p=P))
            onecol = cst.tile([P, 1], BF)
            nc.vector.memset(onecol[:], 1.0)
            of32 = cst.tile([65, 64], F32)
            nc.vector.memset(of32[64:65, :], 1.0)
            ones_r = cst.tile([65, 64], F32R)
            nc.vector.tensor_copy(ones_r[64:65, :], of32[64:65, :])

            # ---------- big SBUF residents (DMA priority order) ----------
            xT = bigio.tile([P, KO, S], BF, tag="xT")
            for ko in range(KO):
                nc.sync.dma_start(xT[:, ko, :], xT_v[:, ko, :])
            wqk_sb = bigio.tile([P, KO, 2 * D], BF, tag="wqk")
            # quarters 0 (q m-tiles 0-3) and 2 (k m-tiles 8-11) first: they
            # unblock qk tiles 0,1 for the pair-0 scores
            for q in (0, 2):
                for ko in range(KO):
                    nc.sync.dma_start(
                        wqk_sb[:, ko, q * 512:(q + 1) * 512],
                        wqkT_v[:, ko, q * 512:(q + 1) * 512])
            wv_sb = bigio.tile([P, KO, D], BF, tag="wv")
            for ko in range(KO):
                nc.sync.dma_start(wv_sb[:, ko, 0:512], wvT_v[:, ko, 0:512])
            for q in (1, 3):
                for ko in range(KO):
                    nc.sync.dma_start(
                        wqk_sb[:, ko, q * 512:(q + 1) * 512],
                        wqkT_v[:, ko, q * 512:(q + 1) * 512])
            for ko in range(KO):
                nc.sync.dma_start(wv_sb[:, ko, 512:1024],
                                  wvT_v[:, ko, 512:1024])
            wp_sb = bigio.tile([P, KO, D], BF, tag="wp")
            for ko in range(KO):
                nc.sync.dma_start(wp_sb[:, ko, :], wpT_v[:, ko, :])
            beffb = cst.tile([P, D], F32)
            nc.sync.dma_start(beffb[:], beffb_d)

            outT = bigio.tile([P, KO, S], BF, tag="outT")
            v_sb = vpool.tile([P, ST, H * (HD + 1)], BF)
            v_hview = v_sb[:].rearrange("p st (h c) -> p st h c", c=HD + 1)
            nc.vector.tensor_copy(
                v_hview[:, :, :, HD:HD + 1],
                onecol[:, None, None, :].broadcast_to([P, ST, H, 1]))

            qk_tiles = {}    # j -> [128, 2, S] tile (0=q, 1=k)

            # ---------- QKV work chains (each: 8 matmuls, 1 PSUM bank) ----
            def qk_chain(j, part, nn):   # part 0=q (m-tile j), 1=k (8+j)
                def go():
                    m = j if part == 0 else NPAIRS + j
                    t = qk_tiles[j]
                    ps = psQ.tile([P, 512], F32, tag="psq",
                                  name=f"qkps{m}_{nn}")
                    for ko in range(KO):
                        nc.tensor.matmul(
                            ps[:], wqk_sb[:, ko, m * P:(m + 1) * P],
                            xT[:, ko, nn * 512:(nn + 1) * 512],
                            start=(ko == 0), stop=(ko == KO - 1))
                    nc.vector.tensor_scalar_add(
                        t[:, part, nn * 512:(nn + 1) * 512], ps[:],
                        bqk_sb[:, m:m + 1])
                return go

            def qk_alloc(j):
                qk_tiles[j] = qkp.tile([P, 2, S], BF, tag="qkt",
                                       name=f"qk{j}")
                return [qk_chain(j, part, nn)
                        for part in (0, 1) for nn in (0, 1)]

            def v_chain(st, nE):
                def go():
                    ps = psQ.tile([P, 512], F32, tag="psq",
                                  name=f"vps{nE}_{st}")
                    for ko in range(KO):
                        nc.tensor.matmul(
                            ps[:], xT[:, ko, st * P:(st + 1) * P],
                            wv_sb[:, ko, nE * 512:(nE + 1) * 512],
                            start=(ko == 0), stop=(ko == KO - 1))
                    nc.vector.tensor_copy(
                        v_hview[:, st, 8 * nE:8 * (nE + 1), 0:HD],
                        ps[:].rearrange("p (h c) -> p h c", c=HD))
                return go

            # ---------- attention ----------
            pend = {}

            def scores_exp(j, m):
                # emit the two heads' matmuls adjacently per chunk so they
                # run concurrently on PE row groups 0-63 / 64-127
                qk_t = qk_tiles[j]
                w = S - m * P
                ats = []
                for hb, base in ((0, 0), (1, 64)):
                    at = attnp.tile([P, S], BF, tag="at",
                                    name=f"at{j}_{hb}_{m}")
                    pend[(j, hb, m)] = at
                    ats.append(at)
                off = m * P
                for cw in _score_chunks(w):
                    pss = []
                    for hb, base in ((0, 0), (1, 64)):
                        ps = psS.tile([P, 512], F32, tag="ps",
                                      name=f"sps{j}_{hb}_{m}")
                        nc.tensor.matmul(
                            ps[:, 0:cw],
                            qk_t[base:base + 64, 1, m * P:(m + 1) * P],
                            qk_t[base:base + 64, 0, off:off + cw],
                            start=True, stop=True)
                        pss.append(ps)
                    for hb in (0, 1):
                        nc.scalar.activation(
                            ats[hb][:, off:off + cw], pss[hb][:, 0:cw],
                            AF.Exp, scale=0.125)
                    off += cw
                for hb in (0, 1):
                    nc.vector.tensor_mul(
                        ats[hb][:, m * P:(m + 1) * P],
                        ats[hb][:, m * P:(m + 1) * P], umask[:])

            def av_m(j, m):
                # narrowed to the causally-nonzero range of each 512 chunk;
                # partial-width accumulate is element-wise legal on HW
                st8 = pend[f"ps{j}"]
                for hb in (0, 1):
                    h = 2 * j + hb
                    at = pend[(j, hb, m)]
                    for n in range(2):
                        lo = max(n * 512, m * P)
                        hi = (n + 1) * 512
                        if lo >= hi:
                            continue
                        nc.tensor.matmul(
                            st8[hb][:, lo:hi],
                            v_sb[:, m, h * (HD + 1):(h + 1) * (HD + 1)],
                            at[:, lo:hi],
                            start=(m == 0), stop=(m == 4 * n + 3),
                            skip_group_check=True)

            def evict_recip(j):
                # move [65,S] AV accumulators out of PSUM so the next pair's
                # AV matmuls get the banks; reciprocal of den row in fp32
                avcs, recs = [], []
                for hb in (0, 1):
                    avc = avsbp.tile([65, S], F32, tag="avc",
                                     name=f"avc{j}_{hb}")
                    nc.vector.tensor_copy(avc[:], pend[f"ps{j}"][hb][:])
                    avcs.append(avc)
                    rt = rtp.tile([65, S], F32R, tag="rt")
                    rt32 = rtp.tile([65, S], F32, tag="rt32", bufs=1)
                    # custom-DVE op misbehaves on single-partition APs on HW:
                    # run over all 65 rows, consume only the den row (64)
                    nc.vector.reciprocal_approx_fast(rt32[:], avc[:])
                    nc.vector.tensor_copy(rt[64:65, :], rt32[64:65, :])
                    recs.append(rt)
                pend[f"avc{j}"] = avcs
                pend[f"rec{j}"] = recs
                del pend[f"ps{j}"]

            def rb_norm(j):
                for hb in (0, 1):
                    rt = pend[f"rec{j}"][hb]
                    avc = pend[f"avc{j}"][hb]
                    tmp = None
                    if hb == 1:
                        tmp = toddp.tile([64, S], BF, tag="todd")
                    for c in range(2):
                        rps = psS.tile([64, 512], F32, tag="ps",
                                       name=f"rbps{j}_{hb}_{c}")
                        nc.tensor.matmul(
                            rps[:], ones_r[64:65, :],
                            rt[64:65, c * 512:(c + 1) * 512],
                            start=True, stop=True)
                        if hb == 0:
                            nc.vector.tensor_mul(
                                outT[0:64, j, c * 512:(c + 1) * 512],
                                avc[0:64, c * 512:(c + 1) * 512], rps[:])
                        else:
                            # DVE lanes cannot shift partitions: multiply to
                            # SBUF tmp, DMA-shift rows 0..63 -> 64..127
                            nc.vector.tensor_mul(
                                tmp[:, c * 512:(c + 1) * 512],
                                avc[0:64, c * 512:(c + 1) * 512], rps[:])
                    if hb == 1:
                        nc.sync.dma_start(outT[64:128, j, :], tmp[:])
                del pend[f"avc{j}"], pend[f"rec{j}"]

            def proj_evict(ps, st, nE):
                ystg = ystgp.tile([P, 512], F32, tag="ystg",
                                  name=f"ystg{st}")
                nc.vector.tensor_add(
                    ystg[:], ps[:], beffb[:, nE * 512:(nE + 1) * 512])
                nc.sync.dma_start(
                    y_d[st * P:(st + 1) * P, nE * 512:(nE + 1) * 512],
                    ystg[:])

            # ---------- interleaved emission ----------
            # prologue: qk for pairs 0,1 and v half 0
            for c in qk_alloc(0):
                c()
            for c in qk_alloc(1):
                c()
            for st in range(ST):
                v_chain(st, 0)()
            vwork = [v_chain(st, 1) for st in range(ST)]

            for j in range(NPAIRS):
                work = []
                if j + 2 < NPAIRS:
                    work.extend(qk_alloc(j + 2))
                if j < 4 and vwork:
                    work.append(vwork.pop(0))
                    work.append(vwork.pop(0))
                for m in range(ST):
                    scores_exp(j, m)
                    if m == 4 and j > 0:
                        rb_norm(j - 1)
                    if m == 0:
                        pend[f"ps{j}"] = [
                            psAV.tile([65, S], F32, tag="av",
                                      name=f"av{j}_{hb}") for hb in range(2)]
                    if m >= 2:
                        av_m(j, m - 2)
                    if work:
                        work.pop(0)()
                av_m(j, ST - 2)
                while work:
                    work.pop(0)()
                av_m(j, ST - 1)
                evict_recip(j)
            rb_norm(NPAIRS - 1)

            # ---------- output projection ----------
            groups = [((0, 1, 2), 0), ((0, 1, 2), 1), ((3, 4, 5), 0),
                      ((3, 4, 5), 1), ((6, 7), 0), ((6, 7), 1)]
            for sts, nE in groups:
                pss = {st: psS.tile([P, 512], F32, tag="ps",
                                    name=f"yps{st}") for st in sts}
                for ko in range(KO):
                    for st in sts:
                        nc.tensor.matmul(
                            pss[st][:],
                            outT[:, ko, st * P:(st + 1) * P],
                            wp_sb[:, ko, nE * 512:(nE + 1) * 512],
                            start=(ko == 0), stop=(ko == KO - 1))
                for st in sts:
                    proj_evict(pss[st], st, nE)

    nc.compile()
    return nc


def kernel(x, w_attn, b_attn, w_proj, b_proj):
    import concourse.bass_utils as bass_utils
    import ml_dtypes

    if "nc" not in _CACHE:
        _CACHE["nc"] = _build()
    nc = _CACHE["nc"]

    bf16 = ml_dtypes.bfloat16
    x = np.asarray(x, dtype=np.float32)
    w_attn = np.asarray(w_attn, dtype=np.float32)
    b_attn = np.asarray(b_attn, dtype=np.float32)
    w_proj = np.asarray(w_proj, dtype=np.float32)
    b_proj = np.asarray(b_proj, dtype=np.float32)

    xT = np.ascontiguousarray(
        np.transpose(x, (0, 2, 1))).astype(bf16)                 # [B, D, S]
    wqkT = np.ascontiguousarray(w_attn[:2 * D].T).astype(bf16)   # [D, 2D]
    wvT = np.ascontiguousarray(w_attn[2 * D:].T).astype(bf16)    # [D, D]
    wpT = np.ascontiguousarray(w_proj.T).astype(bf16)            # [D, D]
    bqk = np.ascontiguousarray(b_attn[:2 * D])
    bv = b_attn[2 * D:]
    beff = (b_proj.astype(np.float64)
            + w_proj.astype(np.float64) @ bv.astype(np.float64)
            ).astype(np.float32)
    beffb = np.ascontiguousarray(np.broadcast_to(beff, (P, D)))
    umask = np.triu(np.ones((P, P), dtype=np.float32)).astype(bf16)

    in_maps = [
        dict(xT=xT[b], wqkT=wqkT, wvT=wvT, wpT=wpT, bqk=bqk, beffb=beffb,
             umask=umask)
        for b in range(B)
    ]
    res = bass_utils.run_bass_kernel_spmd(
        nc, in_maps, core_ids=list(range(NCORES)), trace=TRACE)
    if TRACE:
        _CACHE["exec_time_ns"] = res.exec_time_ns
        _CACHE["trace"] = res.instructions_and_trace
    return np.stack([res.results[b]["y"] for b in range(B)], axis=0)
